# revision 6
# baseline (speedup 1.0000x reference)
"""Multi-Head Latent Attention (MLA) Trainium2 Bass kernel.

Problem: B=4, T=2048, D=1024, H=16 heads x 64, latent 256, causal.
Sharding: (batch, head-half) -> 8 cores. Core c handles batch c//2 and
heads [(c%2)*8, (c%2)*8+8). Each core computes its batch's latents
(duplicated across the 2 cores sharing a batch), its 8 heads' q/k/v,
causal attention, and a row-parallel slice of the output projection.
The host sums the two partial projections per batch (avoids on-device
collectives) and transposes back.

Device-side layouts (per core):
  xT   [1024, 2048]  x^T          (host pre-transposed)
  latT [256, 2048]   latents^T    (2 SBUF tiles of 128 rows)
  qT/kT[512, 2048]   per-head head_dim-on-partitions (4 tiles)
  v1   [128,16,8,65] v in token-on-partition layout + ones column
                     (ones give softmax denominators for free in PV)
  attention computed as S^T [tk, tq]: exp on ScalarE (logits are
  provably tiny -> no max subtraction), multiplicative tri mask,
  PV accumulates [65, 1024] in PSUM (row 64 = denominator).
  outT [512, 2048] normalized concat-head output^T
  partialT [1024, 2048] = Wout_slice^T-stationary matmuls.

All matmuls stream in float32r (1 cycle/row at N>=256 vs 4 for fp32).
"""

import numpy as np

import concourse.bass as bass
import concourse.mybir as mybir
import concourse.tile as tile
from concourse import bacc, library_config

D, H, HD, DL, B, T = 1024, 16, 64, 256, 4, 2048
NCORES = 8
HPC = H // 2            # heads per core
SCALE = HD ** -0.5
F32 = mybir.dt.float32
F32R = mybir.dt.float32r
EXP = mybir.ActivationFunctionType.Exp

_CACHE = {}


def _r(ap):
    return ap


def _emit(tc, nc, t):
    """Emit the per-core program. t: dict of DRAM APs."""
    from contextlib import ExitStack

    with ExitStack() as ctx:
        # ---- persistent tiles (live through attention) ----
        nc.gpsimd.load_library(library_config.attn)
        persist = ctx.enter_context(tc.tile_pool(name="persist", bufs=1))
        qT = [persist.tile([128, T], F32R, tag=f"qT{m}", name=f"qT{m}") for m in range(4)]
        kT = [persist.tile([128, T], F32R, tag=f"kT{m}", name=f"kT{m}") for m in range(4)]
        v1 = persist.tile([128, 16, HPC, HD + 1], F32R, tag="v1")
        tri = persist.tile([128, 128], F32R, tag="tri")
        nc.sync.dma_start(out=tri[:], in_=t["tri"][:, :])
        nc.vector.memset(v1[:, :, :, HD:HD + 1].bitcast(F32), 1.0)

        # ================= stage 1+2: projections =================
        with (
            tc.tile_pool(name="wdown", bufs=1) as wpool,
            tc.tile_pool(name="lat", bufs=1) as latpool,
            tc.tile_pool(name="xin", bufs=2) as xpool,
            tc.tile_pool(name="pj", bufs=3, space="PSUM") as pj,
        ):
            wqd = wpool.tile([128, 8, DL], F32R, tag="wqd")
            wkvd = wpool.tile([128, 8, DL], F32R, tag="wkvd")
            wqup = wpool.tile([128, 2, 512], F32R, tag="wqup")
            wkup = wpool.tile([128, 2, 512], F32R, tag="wkup")
            wvup = wpool.tile([128, 2, 512], F32R, tag="wvup")
            nc.sync.dma_start(out=wqd[:], in_=t["wqd"].rearrange("(a p) c -> p a c", p=128))
            nc.sync.dma_start(out=wkvd[:], in_=t["wkvd"].rearrange("(a p) c -> p a c", p=128))
            nc.sync.dma_start(out=wqup[:], in_=t["wqup"].rearrange("(a p) c -> p a c", p=128))
            nc.sync.dma_start(out=wkup[:], in_=t["wkup"].rearrange("(a p) c -> p a c", p=128))
            nc.sync.dma_start(out=wvup[:], in_=t["wvup"].rearrange("(a p) c -> p a c", p=128))

            latq = [latpool.tile([128, T], F32R, tag=f"lq{m}", name=f"lq{m}") for m in range(2)]
            latkv = [latpool.tile([128, T], F32R, tag=f"lkv{m}", name=f"lkv{m}") for m in range(2)]

            xTv = t["xT"].rearrange("(a p) t -> p a t", p=128)
            for j in range(4):
                js = slice(j * 512, (j + 1) * 512)
                x_sb = xpool.tile([128, 8, 512], F32R, tag="x")
                nc.sync.dma_start(out=x_sb[:], in_=xTv[:, :, js])
                for dst, w in ((latq, wqd), (latkv, wkvd)):
                    for m in range(2):
                        ps = pj.tile([128, 512], F32, tag="ps")
                        for k in range(8):
                            nc.tensor.matmul(
                                ps[:], _r(w[:, k, m * 128:(m + 1) * 128]),
                                _r(x_sb[:, k, :]), start=(k == 0), stop=(k == 7))
                        eng = nc.vector if (m + j) % 2 == 0 else nc.scalar
                        if eng is nc.vector:
                            eng.tensor_copy(dst[m][:, js], ps[:])
                        else:
                            eng.copy(dst[m][:, js], ps[:])
                # q/k up-projections for this token chunk
                for dst, w, lat in ((qT, wqup, latq), (kT, wkup, latkv)):
                    for m in range(4):
                        ps = pj.tile([128, 512], F32, tag="ps")
                        for k in range(2):
                            nc.tensor.matmul(
                                ps[:], _r(w[:, k, m * 128:(m + 1) * 128]),
                                _r(lat[k][:, js]), start=(k == 0), stop=(k == 1))
                        eng = nc.vector if (m + j) % 2 == 0 else nc.scalar
                        if eng is nc.vector:
                            eng.tensor_copy(dst[m][:, js], ps[:])
                        else:
                            eng.copy(dst[m][:, js], ps[:])
                # v for the 4 token blocks of this chunk
                for tb in range(4 * j, 4 * j + 4):
                    ps = pj.tile([128, 512], F32, tag="ps")
                    for k in range(2):
                        nc.tensor.matmul(
                            ps[:], _r(latkv[k][:, tb * 128:(tb + 1) * 128]),
                            _r(wvup[:, k, :]), start=(k == 0), stop=(k == 1))
                    nc.vector.tensor_copy(
                        v1[:, tb, :, 0:HD], ps[:].rearrange("p (h e) -> p h e", h=HPC))

        # ================= stage 3+4: attention + out-proj =================
        with (
            tc.tile_pool(name="attn", bufs=1) as apool,
            tc.tile_pool(name="et", bufs=3) as epool,
            tc.tile_pool(name="misc", bufs=3) as mpool,
            tc.tile_pool(name="osb", bufs=4) as opool,
            tc.tile_pool(name="stp", bufs=2, space="PSUM") as stp,
            tc.tile_pool(name="otp", bufs=1, space="PSUM") as otp,
            tc.tile_pool(name="prp", bufs=2, space="PSUM") as prp,
        ):
            wout = apool.tile([128, 4, D], F32R, tag="wout")
            nc.sync.dma_start(out=wout[:], in_=t["wout"].rearrange("(a p) c -> p a c", p=128))
            outT = [apool.tile([128, T], F32R, tag=f"oT{m}", name=f"oT{m}") for m in range(4)]

            for c in range(2):
                for h in range(HPC):
                    r0 = (h % 2) * 64
                    ot = otp.tile([65, 1024], F32, tag="ot")
                    nd = 8 * (c + 1)
                    # last tk-block writing each PSUM bank of ot (for stop=)
                    lastA = min(nd - 1, 8 * c + 3)
                    lastB = nd - 1
                    for d in range(nd):
                        j0 = max(0, 128 * d - 1024 * c)
                        W = 1024 - j0
                        st = stp.tile([128, 1024], F32, tag="st")
                        kblk = kT[h // 2][r0:r0 + 64, d * 128:(d + 1) * 128]
                        # S^T block: [tk=128, W] split at the psum bank edge
                        for (a, b) in (((0, 512), (512, W)) if W > 512 else ((0, W),)):
                            nc.tensor.matmul(
                                st[:, a:b], _r(kblk),
                                _r(qT[h // 2][r0:r0 + 64,
                                              c * 1024 + j0 + a:c * 1024 + j0 + b]),
                                start=True, stop=True)
                        e = epool.tile([128, 1024], F32R, tag="e")
                        nc.scalar.activation(e[:, 0:W], st[:, 0:W], EXP, scale=SCALE)
                        if 128 * d >= 1024 * c:  # diagonal block -> tri mask
                            nc.vector.tensor_mul(e[:, 0:128], e[:, 0:128], tri[:])
                        # PV accumulate into ot cols [j0, 1024)
                        segs = ((j0, 512), (512, 1024)) if j0 < 512 else ((j0, 1024),)
                        for (a, b) in segs:
                            nc.tensor.matmul(
                                ot[:, a:b], _r(v1[:, d, h, :]), _r(e[:, a - j0:b - j0]),
                                start=(d == 0),
                                stop=(d == (lastA if a < 512 else lastB)))
                    # normalize: rows 0..63 / row 64, broadcast along partitions
                    recip = mpool.tile([1, 1024], F32, tag="recip")
                    nc.vector.reciprocal(recip[:], ot[64:65, :])
                    recipb = mpool.tile([64, 1024], F32, tag="recipb")
                    nc.gpsimd.partition_broadcast(recipb[:], recip[0:1, :], channels=64)
                    nc.vector.tensor_mul(
                        outT[h // 2][r0:r0 + 64, c * 1024:(c + 1) * 1024],
                        ot[0:64, :], recipb[:])
                # out-projection for the two 512-wide chunks now complete
                for c4 in (2 * c, 2 * c + 1):
                    cs = slice(c4 * 512, (c4 + 1) * 512)
                    for m in range(8):
                        ps = prp.tile([128, 512], F32, tag="pr")
                        for k in range(4):
                            nc.tensor.matmul(
                                ps[:], _r(wout[:, k, m * 128:(m + 1) * 128]),
                                _r(outT[k][:, cs]), start=(k == 0), stop=(k == 3))
                        osb = opool.tile([128, 512], F32, tag="osb")
                        eng = nc.vector if (m + c4) % 2 == 0 else nc.scalar
                        if eng is nc.vector:
                            eng.tensor_copy(osb[:], ps[:])
                        else:
                            eng.copy(osb[:], ps[:])
                        nc.sync.dma_start(out=t["outp"][m * 128:(m + 1) * 128, cs], in_=osb[:])


def build_nc():
    nc = bacc.Bacc("TRN2", target_bir_lowering=False, debug=False,
                   enable_asserts=False, num_devices=NCORES)
    t = {
        "xT": nc.dram_tensor("xT", [D, T], F32R, kind="ExternalInput").ap(),
        "wqd": nc.dram_tensor("wqd", [D, DL], F32R, kind="ExternalInput").ap(),
        "wkvd": nc.dram_tensor("wkvd", [D, DL], F32R, kind="ExternalInput").ap(),
        "wqup": nc.dram_tensor("wqup", [DL, 512], F32R, kind="ExternalInput").ap(),
        "wkup": nc.dram_tensor("wkup", [DL, 512], F32R, kind="ExternalInput").ap(),
        "wvup": nc.dram_tensor("wvup", [DL, 512], F32R, kind="ExternalInput").ap(),
        "wout": nc.dram_tensor("wout", [512, D], F32R, kind="ExternalInput").ap(),
        "tri": nc.dram_tensor("tri", [128, 128], F32R, kind="ExternalInput").ap(),
        "outp": nc.dram_tensor("outp", [D, T], F32, kind="ExternalOutput").ap(),
    }
    with tile.TileContext(nc) as tc:
        _emit(tc, nc, t)
    nc.compile()
    return nc


def make_in_maps(inputs):
    x = np.asarray(inputs["x"], np.float32)
    tri = np.triu(np.ones((128, 128), np.float32))
    wqd = np.ascontiguousarray(np.asarray(inputs["Wq_down"], np.float32).T)
    wkvd = np.ascontiguousarray(np.asarray(inputs["Wkv_down"], np.float32).T)
    in_maps = []
    for core in range(NCORES):
        b, hh = core // 2, core % 2
        h0 = hh * HPC * HD
        in_maps.append({
            "xT": np.ascontiguousarray(x[b].T),
            "wqd": wqd,
            "wkvd": wkvd,
            "wqup": np.ascontiguousarray(np.asarray(inputs["Wq_up"], np.float32)[h0:h0 + 512].T),
            "wkup": np.ascontiguousarray(np.asarray(inputs["Wk_up"], np.float32)[h0:h0 + 512].T),
            "wvup": np.ascontiguousarray(np.asarray(inputs["Wv_up"], np.float32)[h0:h0 + 512].T),
            "wout": np.ascontiguousarray(np.asarray(inputs["Wout"], np.float32)[:, h0:h0 + 512].T),
            "tri": tri,
        })
    return in_maps


def postprocess(results):
    out = np.empty((B, T, D), np.float32)
    for b in range(B):
        out[b] = (results[2 * b]["outp"] + results[2 * b + 1]["outp"]).T
    return out


def _get_nc():
    if "nc" not in _CACHE:
        _CACHE["nc"] = build_nc()
    return _CACHE["nc"]


def kernel(**inputs):
    from concourse.bass_utils import run_bass_kernel_spmd
    nc = _get_nc()
    res = run_bass_kernel_spmd(nc, make_in_maps(inputs), core_ids=list(range(NCORES)))
    return postprocess(res.results)


if __name__ == "__main__":
    nc = build_nc()
    print("compiled OK")


# revision 11
# speedup vs baseline: 181.4820x; 181.4820x over previous
"""Multi-Head Latent Attention (MLA) Trainium2 Bass kernel.

Problem: B=4, T=2048, D=1024, H=16 heads x 64, latent 256, causal.
Sharding: (batch, head-half) -> 8 cores. Core c handles batch c//2 and
heads [(c%2)*8, (c%2)*8+8). Each core computes its batch's latents
(duplicated across the 2 cores sharing a batch), its 8 heads' q/k/v,
causal attention, and a row-parallel slice of the output projection.
The host sums the two partial projections per batch (avoids on-device
collectives) and transposes back.

Device-side layouts (per core):
  xT   [1024, 2048]  x^T          (host pre-transposed)
  latT [256, 2048]   latents^T    (2 SBUF tiles of 128 rows)
  qT/kT[512, 2048]   per-head head_dim-on-partitions (4 tiles)
  v1   [128,16,8,65] v in token-on-partition layout + ones column
                     (ones give softmax denominators for free in PV)
  attention computed as S^T [tk, tq]: exp on ScalarE (logits are
  provably tiny -> no max subtraction), multiplicative tri mask,
  PV accumulates [65, 1024] in PSUM (row 64 = denominator).
  outT [512, 2048] normalized concat-head output^T
  partialT [1024, 2048] = Wout_slice^T-stationary matmuls.

All matmuls stream in float32r (1 cycle/row at N>=256 vs 4 for fp32).
"""

import numpy as np

import concourse.bass as bass
import concourse.mybir as mybir
import concourse.tile as tile
from concourse import bacc, library_config

D, H, HD, DL, B, T = 1024, 16, 64, 256, 4, 2048
NCORES = 8
HPC = H // 2            # heads per core
SCALE = HD ** -0.5
F32 = mybir.dt.float32
F32R = mybir.dt.float32r
EXP = mybir.ActivationFunctionType.Exp

_CACHE = {}


def _variant_tag(reps):
    """Cache-busting tag: the neuron compile cache keys on HLO shapes only
    (not the embedded BIR), so two kernel variants with identical I/O would
    collide and reuse a stale NEFF. A dummy input sized by a hash of the
    emitter source (+ reps) makes each variant's HLO unique."""
    import zlib, inspect
    src = inspect.getsource(_emit_once) + f"reps={reps}"
    return (zlib.crc32(src.encode()) + 131 * reps) % 8191 + 1


def _r(ap):
    return ap


def _emit(tc, nc, t, reps=1):
    """Emit the per-core program. t: dict of DRAM APs.

    reps>1 re-emits the whole program serially (timing amplification only)."""
    with tc.tile_pool(name="vtag", bufs=1) as vpool:
        vt = vpool.tile([1, t["vtag"].shape[1]], F32, tag="vt")
        nc.sync.dma_start(out=vt[:], in_=t["vtag"][:, :])
    for _rep in range(reps):
        _emit_once(tc, nc, t)


def _emit_once(tc, nc, t):
    from contextlib import ExitStack

    with ExitStack() as ctx:
        # ---- persistent tiles (live through attention) ----
        nc.gpsimd.load_library(library_config.attn)
        persist = ctx.enter_context(tc.tile_pool(name="persist", bufs=1))
        qT = [persist.tile([128, T], F32R, tag=f"qT{m}", name=f"qT{m}") for m in range(4)]
        kT = [persist.tile([128, T], F32R, tag=f"kT{m}", name=f"kT{m}") for m in range(4)]
        v1 = persist.tile([128, 16, HPC, HD + 1], F32R, tag="v1")
        tri = persist.tile([128, 128], F32R, tag="tri")
        nc.sync.dma_start(out=tri[:], in_=t["tri"][:, :])
        nc.vector.memset(v1[:, :, :, HD:HD + 1].bitcast(F32), 1.0)

        # ================= stage 1+2: projections =================
        with (
            tc.tile_pool(name="wdown", bufs=1) as wpool,
            tc.tile_pool(name="lat", bufs=1) as latpool,
            tc.tile_pool(name="xin", bufs=2) as xpool,
            tc.tile_pool(name="pj", bufs=3, space="PSUM") as pj,
        ):
            wqd = wpool.tile([128, 8, DL], F32R, tag="wqd")
            wkvd = wpool.tile([128, 8, DL], F32R, tag="wkvd")
            wqup = wpool.tile([128, 2, 512], F32R, tag="wqup")
            wkup = wpool.tile([128, 2, 512], F32R, tag="wkup")
            wvup = wpool.tile([128, 2, 512], F32R, tag="wvup")
            nc.sync.dma_start(out=wqd[:], in_=t["wqd"].rearrange("(a p) c -> p a c", p=128))
            nc.sync.dma_start(out=wkvd[:], in_=t["wkvd"].rearrange("(a p) c -> p a c", p=128))
            nc.sync.dma_start(out=wqup[:], in_=t["wqup"].rearrange("(a p) c -> p a c", p=128))
            nc.sync.dma_start(out=wkup[:], in_=t["wkup"].rearrange("(a p) c -> p a c", p=128))
            nc.sync.dma_start(out=wvup[:], in_=t["wvup"].rearrange("(a p) c -> p a c", p=128))

            latq = [latpool.tile([128, T], F32R, tag=f"lq{m}", name=f"lq{m}") for m in range(2)]
            latkv = [latpool.tile([128, T], F32R, tag=f"lkv{m}", name=f"lkv{m}") for m in range(2)]

            xTv = t["xT"].rearrange("(a p) t -> p a t", p=128)
            for j in range(4):
                js = slice(j * 512, (j + 1) * 512)
                x_sb = xpool.tile([128, 8, 512], F32R, tag="x")
                nc.sync.dma_start(out=x_sb[:], in_=xTv[:, :, js])
                for dst, w in ((latq, wqd), (latkv, wkvd)):
                    for m in range(2):
                        ps = pj.tile([128, 512], F32, tag="ps")
                        for k in range(8):
                            nc.tensor.matmul(
                                ps[:], _r(w[:, k, m * 128:(m + 1) * 128]),
                                _r(x_sb[:, k, :]), start=(k == 0), stop=(k == 7))
                        eng = nc.vector if (m + j) % 2 == 0 else nc.scalar
                        if eng is nc.vector:
                            eng.tensor_copy(dst[m][:, js], ps[:])
                        else:
                            eng.copy(dst[m][:, js], ps[:])
                # q/k up-projections for this token chunk
                for dst, w, lat in ((qT, wqup, latq), (kT, wkup, latkv)):
                    for m in range(4):
                        ps = pj.tile([128, 512], F32, tag="ps")
                        for k in range(2):
                            nc.tensor.matmul(
                                ps[:], _r(w[:, k, m * 128:(m + 1) * 128]),
                                _r(lat[k][:, js]), start=(k == 0), stop=(k == 1))
                        eng = nc.vector if (m + j) % 2 == 0 else nc.scalar
                        if eng is nc.vector:
                            eng.tensor_copy(dst[m][:, js], ps[:])
                        else:
                            eng.copy(dst[m][:, js], ps[:])
                # v for the 4 token blocks of this chunk
                for tb in range(4 * j, 4 * j + 4):
                    ps = pj.tile([128, 512], F32, tag="ps")
                    for k in range(2):
                        nc.tensor.matmul(
                            ps[:], _r(latkv[k][:, tb * 128:(tb + 1) * 128]),
                            _r(wvup[:, k, :]), start=(k == 0), stop=(k == 1))
                    nc.vector.tensor_copy(
                        v1[:, tb, :, 0:HD], ps[:].rearrange("p (h e) -> p h e", h=HPC))

        # ================= stage 3+4: attention + out-proj =================
        with (
            tc.tile_pool(name="attn", bufs=1) as apool,
            tc.tile_pool(name="et", bufs=3) as epool,
            tc.tile_pool(name="misc", bufs=3) as mpool,
            tc.tile_pool(name="osb", bufs=4) as opool,
            tc.tile_pool(name="stp", bufs=2, space="PSUM") as stp,
            tc.tile_pool(name="otp", bufs=1, space="PSUM") as otp,
            tc.tile_pool(name="prp", bufs=2, space="PSUM") as prp,
        ):
            wout = apool.tile([128, 4, D], F32R, tag="wout")
            nc.sync.dma_start(out=wout[:], in_=t["wout"].rearrange("(a p) c -> p a c", p=128))
            outT = [apool.tile([128, T], F32R, tag=f"oT{m}", name=f"oT{m}") for m in range(4)]

            for c in range(2):
                for h in range(HPC):
                    r0 = (h % 2) * 64
                    ot = otp.tile([65, 1024], F32, tag="ot")
                    nd = 8 * (c + 1)
                    # last tk-block writing each PSUM bank of ot (for stop=)
                    lastA = min(nd - 1, 8 * c + 3)
                    lastB = nd - 1
                    for d in range(nd):
                        j0 = max(0, 128 * d - 1024 * c)
                        W = 1024 - j0
                        st = stp.tile([128, 1024], F32, tag="st")
                        kblk = kT[h // 2][r0:r0 + 64, d * 128:(d + 1) * 128]
                        # S^T block: [tk=128, W] split at the psum bank edge
                        for (a, b) in (((0, 512), (512, W)) if W > 512 else ((0, W),)):
                            nc.tensor.matmul(
                                st[:, a:b], _r(kblk),
                                _r(qT[h // 2][r0:r0 + 64,
                                              c * 1024 + j0 + a:c * 1024 + j0 + b]),
                                start=True, stop=True)
                        e = epool.tile([128, 1024], F32R, tag="e")
                        nc.scalar.activation(e[:, 0:W], st[:, 0:W], EXP, scale=SCALE)
                        if 128 * d >= 1024 * c:  # diagonal block -> tri mask
                            nc.vector.tensor_mul(e[:, 0:128], e[:, 0:128], tri[:])
                        # PV accumulate into ot cols [j0, 1024)
                        segs = ((j0, 512), (512, 1024)) if j0 < 512 else ((j0, 1024),)
                        for (a, b) in segs:
                            nc.tensor.matmul(
                                ot[:, a:b], _r(v1[:, d, h, :]), _r(e[:, a - j0:b - j0]),
                                start=(d == 0),
                                stop=(d == (lastA if a < 512 else lastB)))
                    # normalize: rows 0..63 / row 64, broadcast along partitions
                    recip = mpool.tile([1, 1024], F32, tag="recip")
                    nc.vector.reciprocal(recip[:], ot[64:65, :])
                    recipb = mpool.tile([64, 1024], F32, tag="recipb")
                    nc.gpsimd.partition_broadcast(recipb[:], recip[0:1, :], channels=64)
                    nc.vector.tensor_mul(
                        outT[h // 2][r0:r0 + 64, c * 1024:(c + 1) * 1024],
                        ot[0:64, :], recipb[:])
                # out-projection for the two 512-wide chunks now complete
                for c4 in (2 * c, 2 * c + 1):
                    cs = slice(c4 * 512, (c4 + 1) * 512)
                    for m in range(8):
                        ps = prp.tile([128, 512], F32, tag="pr")
                        for k in range(4):
                            nc.tensor.matmul(
                                ps[:], _r(wout[:, k, m * 128:(m + 1) * 128]),
                                _r(outT[k][:, cs]), start=(k == 0), stop=(k == 3))
                        osb = opool.tile([128, 512], F32, tag="osb")
                        eng = nc.vector if (m + c4) % 2 == 0 else nc.scalar
                        if eng is nc.vector:
                            eng.tensor_copy(osb[:], ps[:])
                        else:
                            eng.copy(osb[:], ps[:])
                        nc.sync.dma_start(out=t["outp"][m * 128:(m + 1) * 128, cs], in_=osb[:])


def build_nc(reps=1, timing=False):
    """timing=True: all big tensors become Internal DRAM scratch (garbage
    data, identical compute + DMA traffic) so the axon tunnel ships ~nothing
    per call; wall-time slope over `reps` then isolates HW exec time."""
    nc = bacc.Bacc("TRN2", target_bir_lowering=False, debug=False,
                   enable_asserts=False, num_devices=NCORES)
    kin = "Internal" if timing else "ExternalInput"
    kout = "Internal" if timing else "ExternalOutput"
    t = {
        "xT": nc.dram_tensor("xT", [D, T], F32R, kind=kin).ap(),
        "wqd": nc.dram_tensor("wqd", [D, DL], F32R, kind=kin).ap(),
        "wkvd": nc.dram_tensor("wkvd", [D, DL], F32R, kind=kin).ap(),
        "wqup": nc.dram_tensor("wqup", [DL, 512], F32R, kind=kin).ap(),
        "wkup": nc.dram_tensor("wkup", [DL, 512], F32R, kind=kin).ap(),
        "wvup": nc.dram_tensor("wvup", [DL, 512], F32R, kind=kin).ap(),
        "wout": nc.dram_tensor("wout", [512, D], F32R, kind=kin).ap(),
        "tri": nc.dram_tensor("tri", [128, 128], F32R, kind=kin).ap(),
        "vtag": nc.dram_tensor("vtag", [1, _variant_tag(reps)], F32, kind="ExternalInput").ap(),
        "outp": nc.dram_tensor("outp", [D, T], F32, kind=kout).ap(),
    }
    if timing:
        t["done"] = nc.dram_tensor("done", [1, 1], F32, kind="ExternalOutput").ap()
    with tile.TileContext(nc) as tc:
        _emit(tc, nc, t, reps=reps)
        if timing:
            with tc.tile_pool(name="donep", bufs=1) as dp:
                dt_ = dp.tile([1, 1], F32, tag="dn")
                nc.vector.memset(dt_[:], 1.0)
                nc.sync.dma_start(out=t["done"][:, :], in_=dt_[:])
    nc.compile()
    return nc


def make_in_maps(inputs, reps=1):
    x = np.asarray(inputs["x"], np.float32)
    tri = np.triu(np.ones((128, 128), np.float32))
    wqd = np.ascontiguousarray(np.asarray(inputs["Wq_down"], np.float32).T)
    wkvd = np.ascontiguousarray(np.asarray(inputs["Wkv_down"], np.float32).T)
    in_maps = []
    for core in range(NCORES):
        b, hh = core // 2, core % 2
        h0 = hh * HPC * HD
        in_maps.append({
            "xT": np.ascontiguousarray(x[b].T),
            "wqd": wqd,
            "wkvd": wkvd,
            "wqup": np.ascontiguousarray(np.asarray(inputs["Wq_up"], np.float32)[h0:h0 + 512].T),
            "wkup": np.ascontiguousarray(np.asarray(inputs["Wk_up"], np.float32)[h0:h0 + 512].T),
            "wvup": np.ascontiguousarray(np.asarray(inputs["Wv_up"], np.float32)[h0:h0 + 512].T),
            "wout": np.ascontiguousarray(np.asarray(inputs["Wout"], np.float32)[:, h0:h0 + 512].T),
            "tri": tri,
            "vtag": np.zeros((1, _variant_tag(reps)), np.float32),
        })
    return in_maps


def postprocess(results):
    out = np.empty((B, T, D), np.float32)
    for b in range(B):
        out[b] = (results[2 * b]["outp"] + results[2 * b + 1]["outp"]).T
    return out


def _get_nc():
    if "nc" not in _CACHE:
        _CACHE["nc"] = build_nc()
    return _CACHE["nc"]


def kernel(**inputs):
    from concourse.bass_utils import run_bass_kernel_spmd
    nc = _get_nc()
    res = run_bass_kernel_spmd(nc, make_in_maps(inputs), core_ids=list(range(NCORES)))
    return postprocess(res.results)


if __name__ == "__main__":
    nc = build_nc()
    print("compiled OK")


# revision 12
# speedup vs baseline: 308.2845x; 1.6987x over previous
"""Multi-Head Latent Attention (MLA) Trainium2 Bass kernel.

Problem: B=4, T=2048, D=1024, H=16 heads x 64, latent 256, causal.
Sharding: (batch, head-half) -> 8 cores. Core c handles batch c//2 and
heads [(c%2)*8, (c%2)*8+8). Each core computes its batch's latents
(duplicated across the 2 cores sharing a batch), its 8 heads' q/k/v,
causal attention, and a row-parallel slice of the output projection.
The host sums the two partial projections per batch (avoids on-device
collectives) and transposes back.

Device-side layouts (per core):
  xT   [1024, 2048]  x^T          (host pre-transposed)
  latT [256, 2048]   latents^T    (2 SBUF tiles of 128 rows)
  qT/kT[512, 2048]   per-head head_dim-on-partitions (4 tiles)
  v1   [128,16,8,65] v in token-on-partition layout + ones column
                     (ones give softmax denominators for free in PV)
  attention computed as S^T [tk, tq]: exp on ScalarE (logits are
  provably tiny -> no max subtraction), multiplicative tri mask,
  PV accumulates [65, 1024] in PSUM (row 64 = denominator).
  outT [512, 2048] normalized concat-head output^T
  partialT [1024, 2048] = Wout_slice^T-stationary matmuls.

All matmuls stream in float32r (1 cycle/row at N>=256 vs 4 for fp32).
"""

import numpy as np

import concourse.bass as bass
import concourse.mybir as mybir
import concourse.tile as tile
from concourse import bacc, library_config

D, H, HD, DL, B, T = 1024, 16, 64, 256, 4, 2048
NCORES = 8
HPC = H // 2            # heads per core
SCALE = HD ** -0.5
F32 = mybir.dt.float32
F32R = mybir.dt.float32r
EXP = mybir.ActivationFunctionType.Exp

_CACHE = {}


def _variant_tag(reps):
    """Cache-busting tag: the neuron compile cache keys on HLO shapes only
    (not the embedded BIR), so two kernel variants with identical I/O would
    collide and reuse a stale NEFF. A dummy input sized by a hash of the
    emitter source (+ reps) makes each variant's HLO unique."""
    import zlib, inspect
    src = inspect.getsource(_emit_once) + f"reps={reps}"
    return (zlib.crc32(src.encode()) + 131 * reps) % 8191 + 1


def _r(ap):
    return ap


def _emit(tc, nc, t, reps=1):
    """Emit the per-core program. t: dict of DRAM APs.

    reps>1 re-emits the whole program serially (timing amplification only)."""
    with tc.tile_pool(name="vtag", bufs=1) as vpool:
        vt = vpool.tile([1, t["vtag"].shape[1]], F32, tag="vt")
        nc.sync.dma_start(out=vt[:], in_=t["vtag"][:, :])
    for _rep in range(reps):
        _emit_once(tc, nc, t)


def _emit_once(tc, nc, t):
    from contextlib import ExitStack

    with ExitStack() as ctx:
        # ---- persistent tiles (live through attention) ----
        nc.gpsimd.load_library(library_config.attn)
        persist = ctx.enter_context(tc.tile_pool(name="persist", bufs=1))
        qT = [persist.tile([128, T], F32R, tag=f"qT{m}", name=f"qT{m}") for m in range(4)]
        kT = [persist.tile([128, T], F32R, tag=f"kT{m}", name=f"kT{m}") for m in range(4)]
        v1 = persist.tile([128, 16, HPC, HD + 1], F32R, tag="v1")
        tri = persist.tile([128, 128], F32R, tag="tri")
        nc.sync.dma_start(out=tri[:], in_=t["tri"][:, :])
        nc.vector.memset(v1[:, :, :, HD:HD + 1].bitcast(F32), 1.0)

        # ================= stage 1+2: projections =================
        with (
            tc.tile_pool(name="wdown", bufs=1) as wpool,
            tc.tile_pool(name="lat", bufs=1) as latpool,
            tc.tile_pool(name="xin", bufs=2) as xpool,
            tc.tile_pool(name="pj", bufs=3, space="PSUM") as pj,
        ):
            wqd = wpool.tile([128, 8, DL], F32R, tag="wqd")
            wkvd = wpool.tile([128, 8, DL], F32R, tag="wkvd")
            wqup = wpool.tile([128, 2, 512], F32R, tag="wqup")
            wkup = wpool.tile([128, 2, 512], F32R, tag="wkup")
            wvup = wpool.tile([128, 2, 512], F32R, tag="wvup")
            nc.sync.dma_start(out=wqd[:], in_=t["wqd"].rearrange("(a p) c -> p a c", p=128))
            nc.sync.dma_start(out=wkvd[:], in_=t["wkvd"].rearrange("(a p) c -> p a c", p=128))
            nc.sync.dma_start(out=wqup[:], in_=t["wqup"].rearrange("(a p) c -> p a c", p=128))
            nc.sync.dma_start(out=wkup[:], in_=t["wkup"].rearrange("(a p) c -> p a c", p=128))
            nc.sync.dma_start(out=wvup[:], in_=t["wvup"].rearrange("(a p) c -> p a c", p=128))

            latq = [latpool.tile([128, T], F32R, tag=f"lq{m}", name=f"lq{m}") for m in range(2)]
            latkv = [latpool.tile([128, T], F32R, tag=f"lkv{m}", name=f"lkv{m}") for m in range(2)]

            xTv = t["xT"].rearrange("(a p) t -> p a t", p=128)
            for j in range(4):
                js = slice(j * 512, (j + 1) * 512)
                x_sb = xpool.tile([128, 8, 512], F32R, tag="x")
                nc.sync.dma_start(out=x_sb[:], in_=xTv[:, :, js])
                for dst, w in ((latq, wqd), (latkv, wkvd)):
                    for m in range(2):
                        ps = pj.tile([128, 512], F32, tag="ps")
                        for k in range(8):
                            nc.tensor.matmul(
                                ps[:], _r(w[:, k, m * 128:(m + 1) * 128]),
                                _r(x_sb[:, k, :]), start=(k == 0), stop=(k == 7))
                        eng = nc.vector if (m + j) % 2 == 0 else nc.scalar
                        if eng is nc.vector:
                            eng.tensor_copy(dst[m][:, js], ps[:])
                        else:
                            eng.copy(dst[m][:, js], ps[:])
                # q/k up-projections for this token chunk
                for dst, w, lat in ((qT, wqup, latq), (kT, wkup, latkv)):
                    for m in range(4):
                        ps = pj.tile([128, 512], F32, tag="ps")
                        for k in range(2):
                            nc.tensor.matmul(
                                ps[:], _r(w[:, k, m * 128:(m + 1) * 128]),
                                _r(lat[k][:, js]), start=(k == 0), stop=(k == 1))
                        eng = nc.vector if (m + j) % 2 == 0 else nc.scalar
                        if eng is nc.vector:
                            eng.tensor_copy(dst[m][:, js], ps[:])
                        else:
                            eng.copy(dst[m][:, js], ps[:])
                # v for the 4 token blocks of this chunk
                for tb in range(4 * j, 4 * j + 4):
                    ps = pj.tile([128, 512], F32, tag="ps")
                    for k in range(2):
                        nc.tensor.matmul(
                            ps[:], _r(latkv[k][:, tb * 128:(tb + 1) * 128]),
                            _r(wvup[:, k, :]), start=(k == 0), stop=(k == 1))
                    nc.vector.tensor_copy(
                        v1[:, tb, :, 0:HD], ps[:].rearrange("p (h e) -> p h e", h=HPC))

        # ================= stage 3+4: attention + out-proj =================
        with (
            tc.tile_pool(name="attn", bufs=1) as apool,
            tc.tile_pool(name="et", bufs=3) as epool,
            tc.tile_pool(name="misc", bufs=3) as mpool,
            tc.tile_pool(name="osb", bufs=4) as opool,
            tc.tile_pool(name="stp", bufs=2, space="PSUM") as stp,
            tc.tile_pool(name="otp", bufs=2, space="PSUM") as otp,
        ):
            wout = apool.tile([128, 4, D], F32R, tag="wout")
            nc.sync.dma_start(out=wout[:], in_=t["wout"].rearrange("(a p) c -> p a c", p=128))
            outT = [apool.tile([128, T], F32R, tag=f"oT{m}", name=f"oT{m}") for m in range(4)]

            def pv(ot, h, d, e, j0, nd, lastA, lastB):
                # PV accumulate into ot cols [j0, 1024)
                segs = ((j0, 512), (512, 1024)) if j0 < 512 else ((j0, 1024),)
                for (a, b) in segs:
                    nc.tensor.matmul(
                        ot[:, a:b], _r(v1[:, d, h, :]), _r(e[:, a - j0:b - j0]),
                        start=(d == 0),
                        stop=(d == (lastA if a < 512 else lastB)))

            for c in range(2):
                for h in range(HPC):
                    r0 = (h % 2) * 64
                    ot = otp.tile([65, 1024], F32, tag="ot")
                    nd = 8 * (c + 1)
                    # last tk-block writing each PSUM bank of ot (for stop=)
                    lastA = min(nd - 1, 8 * c + 3)
                    lastB = nd - 1
                    pend = None   # software-pipeline: PV trails ST/exp by one
                    for d in range(nd):
                        j0 = max(0, 128 * d - 1024 * c)
                        W = 1024 - j0
                        st = stp.tile([128, 1024], F32, tag="st")
                        kblk = kT[h // 2][r0:r0 + 64, d * 128:(d + 1) * 128]
                        # S^T block: [tk=128, W] split at the psum bank edge
                        for (a, b) in (((0, 512), (512, W)) if W > 512 else ((0, W),)):
                            nc.tensor.matmul(
                                st[:, a:b], _r(kblk),
                                _r(qT[h // 2][r0:r0 + 64,
                                              c * 1024 + j0 + a:c * 1024 + j0 + b]),
                                start=True, stop=True)
                        e = epool.tile([128, 1024], F32R, tag="e")
                        nc.scalar.activation(e[:, 0:W], st[:, 0:W], EXP, scale=SCALE)
                        if 128 * d >= 1024 * c:  # diagonal block -> tri mask
                            nc.vector.tensor_mul(e[:, 0:128], e[:, 0:128], tri[:])
                        if pend is not None:
                            pv(ot, h, *pend)
                        pend = (d, e, j0, nd, lastA, lastB)
                    pv(ot, h, *pend)
                    # normalize: rows 0..63 / row 64, broadcast along partitions
                    recip = mpool.tile([1, 1024], F32, tag="recip")
                    nc.vector.reciprocal(recip[:], ot[64:65, :])
                    recipb = mpool.tile([64, 1024], F32, tag="recipb")
                    nc.gpsimd.partition_broadcast(recipb[:], recip[0:1, :], channels=64)
                    nc.vector.tensor_mul(
                        outT[h // 2][r0:r0 + 64, c * 1024:(c + 1) * 1024],
                        ot[0:64, :], recipb[:])
                # out-projection for the two 512-wide chunks now complete
                for c4 in (2 * c, 2 * c + 1):
                    cs = slice(c4 * 512, (c4 + 1) * 512)
                    for m in range(8):
                        ps = stp.tile([128, 512], F32, tag="st", name="prps")
                        for k in range(4):
                            nc.tensor.matmul(
                                ps[:], _r(wout[:, k, m * 128:(m + 1) * 128]),
                                _r(outT[k][:, cs]), start=(k == 0), stop=(k == 3))
                        osb = opool.tile([128, 512], F32, tag="osb")
                        nc.vector.tensor_copy(osb[:], ps[:])
                        nc.sync.dma_start(out=t["outp"][m * 128:(m + 1) * 128, cs], in_=osb[:])


def build_nc(reps=1, timing=False):
    """timing=True: all big tensors become Internal DRAM scratch (garbage
    data, identical compute + DMA traffic) so the axon tunnel ships ~nothing
    per call; wall-time slope over `reps` then isolates HW exec time."""
    nc = bacc.Bacc("TRN2", target_bir_lowering=False, debug=False,
                   enable_asserts=False, num_devices=NCORES)
    kin = "Internal" if timing else "ExternalInput"
    kout = "Internal" if timing else "ExternalOutput"
    t = {
        "xT": nc.dram_tensor("xT", [D, T], F32R, kind=kin).ap(),
        "wqd": nc.dram_tensor("wqd", [D, DL], F32R, kind=kin).ap(),
        "wkvd": nc.dram_tensor("wkvd", [D, DL], F32R, kind=kin).ap(),
        "wqup": nc.dram_tensor("wqup", [DL, 512], F32R, kind=kin).ap(),
        "wkup": nc.dram_tensor("wkup", [DL, 512], F32R, kind=kin).ap(),
        "wvup": nc.dram_tensor("wvup", [DL, 512], F32R, kind=kin).ap(),
        "wout": nc.dram_tensor("wout", [512, D], F32R, kind=kin).ap(),
        "tri": nc.dram_tensor("tri", [128, 128], F32R, kind=kin).ap(),
        "vtag": nc.dram_tensor("vtag", [1, _variant_tag(reps)], F32, kind="ExternalInput").ap(),
        "outp": nc.dram_tensor("outp", [D, T], F32, kind=kout).ap(),
    }
    if timing:
        t["done"] = nc.dram_tensor("done", [1, 1], F32, kind="ExternalOutput").ap()
    with tile.TileContext(nc) as tc:
        _emit(tc, nc, t, reps=reps)
        if timing:
            with tc.tile_pool(name="donep", bufs=1) as dp:
                dt_ = dp.tile([1, 1], F32, tag="dn")
                nc.vector.memset(dt_[:], 1.0)
                nc.sync.dma_start(out=t["done"][:, :], in_=dt_[:])
    nc.compile()
    return nc


def make_in_maps(inputs, reps=1):
    x = np.asarray(inputs["x"], np.float32)
    tri = np.triu(np.ones((128, 128), np.float32))
    wqd = np.ascontiguousarray(np.asarray(inputs["Wq_down"], np.float32).T)
    wkvd = np.ascontiguousarray(np.asarray(inputs["Wkv_down"], np.float32).T)
    in_maps = []
    for core in range(NCORES):
        b, hh = core // 2, core % 2
        h0 = hh * HPC * HD
        in_maps.append({
            "xT": np.ascontiguousarray(x[b].T),
            "wqd": wqd,
            "wkvd": wkvd,
            "wqup": np.ascontiguousarray(np.asarray(inputs["Wq_up"], np.float32)[h0:h0 + 512].T),
            "wkup": np.ascontiguousarray(np.asarray(inputs["Wk_up"], np.float32)[h0:h0 + 512].T),
            "wvup": np.ascontiguousarray(np.asarray(inputs["Wv_up"], np.float32)[h0:h0 + 512].T),
            "wout": np.ascontiguousarray(np.asarray(inputs["Wout"], np.float32)[:, h0:h0 + 512].T),
            "tri": tri,
            "vtag": np.zeros((1, _variant_tag(reps)), np.float32),
        })
    return in_maps


def postprocess(results):
    out = np.empty((B, T, D), np.float32)
    for b in range(B):
        out[b] = (results[2 * b]["outp"] + results[2 * b + 1]["outp"]).T
    return out


def _get_nc():
    if "nc" not in _CACHE:
        _CACHE["nc"] = build_nc()
    return _CACHE["nc"]


def kernel(**inputs):
    from concourse.bass_utils import run_bass_kernel_spmd
    nc = _get_nc()
    res = run_bass_kernel_spmd(nc, make_in_maps(inputs), core_ids=list(range(NCORES)))
    return postprocess(res.results)


if __name__ == "__main__":
    nc = build_nc()
    print("compiled OK")
